# revision 22
# baseline (speedup 1.0000x reference)
"""Trainium2 Bass kernel for nn_BigNetwork (32 parallel Linear(4,1) heads).

Computes out[k, n, 0] = dot(x[n, :], W[k, 0, :]) + b[k, 0] for
x [2_000_000, 4] f32, W [32, 1, 4] f32, b [32, 1] f32 -> out [32, 2_000_000, 1] f32.

Strategy (data-parallel over 8 NeuronCores, x sharded along N):
  Per core (250_000 rows), iterate x-tiles of 16 row-groups x Fi rows:
    1. Strided DMA loads bring x rows in "pre-transpose" layout
       lx[pp, 32G+4a+d] = x[base + (4G+a)*Fi + m0 + pp, d]  (16B chunks).
    2. PE transpose -> T[32G+4a+d, p] = x[base + (4G+a)*Fi + p, d] in PSUM,
       copied to SBUF.  K-groups of 16 sit at 32-aligned partitions.
    3. Four K=16 matmuls with a block-diagonal replicated weight lhsT:
       psum_G[32a+k, p] = dot(x[base+(4G+a)*Fi+p, :], W[k]) .
    4. Bias-add copies PSUM -> SBUF staging S[32a+k, G*Fi+p] (ACT/DVE split).
    5. One large DMA stores S to out[k, n] with 4*Fi-byte-contiguous runs.
  Output per core is o[32, 250_000]; host concatenates along n.
"""

import sys

if "/opt/trn_rl_repo" not in sys.path:
    sys.path.insert(0, "/opt/trn_rl_repo")

import numpy as np

from concourse import bass, mybir
import concourse.bacc as bacc
from concourse.tile import TileContext
from concourse.tile_rust import add_dep_helper
from concourse.bass_utils import run_bass_kernel_spmd

N_CORES = 8
N_TOTAL = 2_000_000
NC_ROWS = N_TOTAL // N_CORES  # 250_000
KHEADS = 32
D = 4
JG = 16  # j-groups (of Fi rows each) per x-tile
# 30 full tiles of 16*512 rows + one tail tile of 16*265 rows = 250_000
TILE_FS = [512] * 30 + [265]
assert JG * sum(TILE_FS) == NC_ROWS

F32 = mybir.dt.float32


def _build_bass(tile_fs=None, nc_rows=None):
    tile_fs = TILE_FS if tile_fs is None else tile_fs
    nc_rows = NC_ROWS if nc_rows is None else nc_rows
    assert JG * sum(tile_fs) == nc_rows
    # Group equal-Fi tiles (5 per group for Fi=512) so stores amortize;
    # uneven/tail tiles go in singleton groups.
    tile_groups = []
    i = 0
    while i < len(tile_fs):
        if tile_fs[i] == 512:
            j = i
            while j < len(tile_fs) and tile_fs[j] == 512 and j - i < 5:
                j += 1
            tile_groups.append(tile_fs[i:j])
            i = j
        else:
            tile_groups.append([tile_fs[i]])
            i += 1
    nc = bacc.Bacc("TRN2", target_bir_lowering=False)
    x = nc.dram_tensor("x", [nc_rows, D], F32, kind="ExternalInput")
    wrep = nc.dram_tensor("wrep", [128, 128], F32, kind="ExternalInput")
    bvec = nc.dram_tensor("bvec", [128, 1], F32, kind="ExternalInput")
    ident = nc.dram_tensor("ident", [128, 128], F32, kind="ExternalInput")
    o = nc.dram_tensor("o", [KHEADS, nc_rows], F32, kind="ExternalOutput")

    with TileContext(nc) as tc:
        with (
            tc.tile_pool(name="consts", bufs=1) as cpool,
            tc.tile_pool(name="lxp", bufs=8) as lxpool,
            tc.tile_pool(name="tp", bufs=3) as tpool,
            tc.tile_pool(name="sp", bufs=3) as spool,
            tc.tile_pool(name="pst", bufs=3, space="PSUM") as ptpool,
            tc.tile_pool(name="pso", bufs=4, space="PSUM") as popool,
            tc.tile_pool(name="psd", bufs=1, space="PSUM") as psdpool,
        ):
            w_sb = cpool.tile([128, 128], F32, name="w_sb")
            nc.sync.dma_start(w_sb, wrep[:, :])
            b_sb = cpool.tile([128, 1], F32, name="b_sb")
            nc.sync.dma_start(b_sb, bvec[:, :])
            id_sb = cpool.tile([128, 128], F32, name="id_sb")
            nc.sync.dma_start(id_sb, ident[:, :])

            # Relay consts through GPSIMD so a single dummy PE transpose
            # (one Pool-sem wait) brings PE's vector clock past all const
            # loads: real PE instructions then need at most one sync wait
            # (walrus rejects multi-wait transpose/LDW instructions).
            w_cp = cpool.tile([128, 128], F32, name="w_cp")
            nc.gpsimd.tensor_copy(w_cp[:, :], w_sb[:, :])
            id_cp = cpool.tile([128, 128], F32, name="id_cp")
            nc.gpsimd.tensor_copy(id_cp[:, :], id_sb[:, :])
            dummy_ps = psdpool.tile([128, 128], F32, name="dummy_ps")
            nc.tensor.transpose(dummy_ps[:, :], w_cp[:, :], id_cp[:, :])

            base = 0
            prev_mm = None
            dma_engines = [nc.sync, nc.scalar]
            dma_i = 0
            for tile_group in tile_groups:
                g = len(tile_group)
                gbase = base
                s_tile = spool.tile([128, 4 * sum(tile_group)], F32, name="s_tile", tag="s")
                for t, Fi in enumerate(tile_group):
                    t_sb = tpool.tile([128, Fi], F32, name="t_sb", tag="t")
                    ps_t = ptpool.tile([128, Fi], F32, name="ps_t", tag="pt")
                    # 1x1 dummy write absorbs the PSUM-slot drain-wait (PE
                    # self sem) so each real transpose carries only its Pool
                    # wait.  Pinned after the previous tile's matmuls so PE's
                    # vector clock already covers the DVE slot-release.
                    dmy = nc.tensor.transpose(
                        ps_t[0:1, 0:1], id_cp[0:1, 0:1], id_cp[0:1, 0:1]
                    )
                    if prev_mm is not None:
                        add_dep_helper(
                            dmy.ins, prev_mm.ins, sync=False, reason="pin dummy"
                        )
                    if Fi == 512:
                        # Merged load: row = base + (16G+4a+m)*128 + pp, so the
                        # whole tile is one 3-dim AP with contiguous dst
                        # lxp_big[pp, (G a m d)].
                        lxp_big = lxpool.tile([128, 256], F32, name="lxp_big", tag="lxb")
                        src = bass.AP(
                            x, base * D, [[D, 128], [128 * D, 64], [1, D]]
                        )
                        dma_engines[dma_i % 2].dma_start(lxp_big[:, :], src)
                        dma_i += 1
                        srcv = lxp_big.rearrange(
                            "p (G a m d) -> p G a m d", G=4, a=4, m=4
                        )
                        for m in range(4):
                            lx = lxpool.tile([128, 128], F32, name="lx", tag="lx")
                            lxv = lx.rearrange(
                                "p (G two a d) -> p G two a d", G=4, two=2, a=4
                            )
                            for h in range(2):
                                nc.gpsimd.tensor_copy(
                                    lxv[:, :, h : h + 1, :, :].squeeze(),
                                    srcv[:, :, :, m : m + 1, :].squeeze(),
                                )
                            nc.tensor.transpose(
                                ps_t[:, m * 128 : (m + 1) * 128], lx[:, :], id_cp[:, :]
                            )
                    else:
                        for m0 in range(0, Fi, 128):
                            bw = min(128, Fi - m0)
                            # Packed per-m load (tail path):
                            # lxp[pp, 16G+4a+d] = x[base+(4G+a)*Fi+m0+pp, d]
                            lxp = lxpool.tile([128, 64], F32, name="lxp", tag="lxp")
                            src = bass.AP(
                                x,
                                (base + m0) * D,
                                [[D, bw], [Fi * D, JG], [1, D]],
                            )
                            dma_engines[dma_i % 2].dma_start(lxp[:bw, :], src)
                            dma_i += 1
                            lx = lxpool.tile([128, 128], F32, name="lx", tag="lx")
                            lxv = lx.rearrange(
                                "p (G two ad) -> p G two ad", G=4, two=2
                            )
                            for h in range(2):
                                nc.gpsimd.tensor_copy(
                                    lxv[:bw, :, h : h + 1, :],
                                    lxp[:bw, :].rearrange(
                                        "p (G one ad) -> p G one ad", G=4, one=1
                                    ),
                                )
                            nc.tensor.transpose(
                                ps_t[:, m0 : m0 + bw], lx[:bw, :], id_cp[:bw, :bw]
                            )
                    if t % 2 == 0:
                        nc.vector.tensor_copy(t_sb[:, :], ps_t[:, :])
                    else:
                        nc.scalar.copy(t_sb[:, :], ps_t[:, :])

                    for G in range(4):
                        ps_o = popool.tile([128, Fi], F32, name="ps_o", tag="po")
                        prev_mm = nc.tensor.matmul(
                            ps_o[:, :],
                            lhsT=w_cp[32 * G : 32 * G + 16, :],
                            rhs=t_sb[32 * G : 32 * G + 16, :],
                            start=True,
                            stop=True,
                            tile_position=(32 * G, 0),
                        )
                        off = (4 * t + G) * Fi
                        dst_s = s_tile[:, off : off + Fi]
                        # Bias-add PSUM->SBUF copies split across DVE and ACT
                        # (Bacc's generate_event_semaphores legalizes any
                        # multi-wait instructions this creates).
                        if G % 2 == 0:
                            nc.vector.tensor_scalar(
                                dst_s, ps_o[:, :], b_sb[:, 0:1], None,
                                mybir.AluOpType.add,
                            )
                        else:
                            nc.scalar.add(dst_s, ps_o[:, :], add=b_sb[:, 0:1])
                    base += JG * Fi
                # o[k, gbase + (16t+4G+a)*Fi + p] <- s_tile[32a+k, (4t+G)*Fi+p]
                # one DMA per a; (t,G) strides merge -> 3-dim dst AP.
                Fi = tile_group[0]
                for a in range(4):
                    odst = bass.AP(
                        o,
                        gbase + a * Fi,
                        [[nc_rows, 32], [4 * Fi, 4 * g], [1, Fi]],
                    )
                    dma_engines[dma_i % 2].dma_start(
                        odst, s_tile[32 * a : 32 * a + 32, :]
                    )
                    dma_i += 1
    nc.compile()
    return nc


_CACHE: dict = {}


def _get_nc():
    if "nc" not in _CACHE:
        _CACHE["nc"] = _build_bass()
    return _CACHE["nc"]


def _prep_weights(W: np.ndarray, b: np.ndarray):
    # wrep[32G + 4a + d, 32a + k] = W[k, 0, d]; zeros elsewhere.
    wrep = np.zeros((128, 128), dtype=np.float32)
    for a in range(4):
        for d in range(D):
            for G in range(4):
                wrep[32 * G + 4 * a + d, 32 * a : 32 * a + 32] = W[:, 0, d]
    # bvec[32a + k] = b[k, 0]
    bvec = np.tile(b[:, 0], 4).reshape(128, 1).astype(np.float32)
    ident = np.eye(128, dtype=np.float32)
    return wrep, bvec, ident


def kernel(x: np.ndarray, W: np.ndarray, b: np.ndarray) -> np.ndarray:
    x = np.ascontiguousarray(x, dtype=np.float32)
    wrep, bvec, ident = _prep_weights(
        np.asarray(W, dtype=np.float32), np.asarray(b, dtype=np.float32)
    )
    nc = _get_nc()
    in_maps = []
    for c in range(N_CORES):
        xs = x[c * NC_ROWS : (c + 1) * NC_ROWS]
        in_maps.append({"x": xs, "wrep": wrep, "bvec": bvec, "ident": ident})
    res = run_bass_kernel_spmd(nc, in_maps, core_ids=list(range(N_CORES)))
    outs = [res.results[c]["o"] for c in range(N_CORES)]
    full = np.concatenate(outs, axis=1)
    return full.reshape(KHEADS, N_TOTAL, 1)


if __name__ == "__main__":
    rng = np.random.default_rng(0)
    x = rng.standard_normal((N_TOTAL, D), dtype=np.float32)
    W = rng.uniform(-0.5, 0.5, (KHEADS, 1, D)).astype(np.float32)
    b = rng.uniform(-0.5, 0.5, (KHEADS, 1)).astype(np.float32)
    out = kernel(x, W, b)
    ref = np.einsum("nd,kod->kno", x, W)[:, :, :] + b[:, None, :]
    err = np.abs(out - ref).max()
    print("absmax err:", err)
